# revision 1
# baseline (speedup 1.0000x reference)
"""DiffusionConv (Chebyshev graph diffusion conv) Trainium2 kernel, 8 NeuronCores.

Math (faithful to the reference's raw reshapes):
  x0 = x.reshape(n, c*b)                         # (10000, 4096)
  for each support s: x1_s = A_s @ x0 ; x2_s = 2*A_s@x1_s - x0
  xcat[b, n_, c*M + m] = xs[m][n_, c*64 + b]     # xs = [x0, x1_0, x2_0, x1_1, x2_1]
  out = xcat @ Theta + bias                      # (64, 10000, 64)

Sharding: the column dim of x0 is (c, b) interleaved; batch b is data-parallel
through the whole computation, so core k owns b in [8k, 8k+8) (512 columns of
x0). Zero inter-core communication. Per core, each spmm is done as a
gather (dma_gather of source rows) + one-hot PE matmul segment-sum into PSUM
per 128-row destination block. The Chebyshev affine (2*A@x1 - x0) and the
(c, m) -> c*M+m interleave of Theta are folded into a host-side rearrangement
of Theta, so the device only computes 4 plain spmms (z_s := A_s @ x1_s) plus a
final dense projection out[b] = Xcat_b @ Theta_hat + bias with
Xcat_b = [x0 | x1_0 | z_0 | x1_1 | z_1] (per-b column slices).
"""

import os
from contextlib import ExitStack

import numpy as np
import ml_dtypes

import concourse.bass as bass
import concourse.bacc as bacc
import concourse.tile as tile
import concourse.mybir as mybir

# ---- problem constants (hardcoded per contest rules) ----
N_NODES = 10000
N_EDGES = 320000
N_SUPPORTS = 2
C_IN = 64
C_OUT = 64
BATCH = 64
NCORES = 8
BLOC = BATCH // NCORES          # 8 batches per core
COLS = C_IN * BLOC              # 512 columns of x0 per core
NBLK = (N_NODES + 127) // 128   # 79 dest row blocks
NPAD = NBLK * 128               # 10112

GATHER_B = 1024                 # edges per dma_gather call (SWDGE ring fits 1024 descs)
TPC = GATHER_B // 128           # matmul tiles per gather call

bf16 = mybir.dt.bfloat16
f32 = mybir.dt.float32
i16 = mybir.dt.int16

LAST_RESULT = {}


# --------------------------------------------------------------------------
# host-side edge preprocessing (shared across all cores)
# --------------------------------------------------------------------------
def _prep_edges(rows, cols, vals):
    """Per dest 128-block: dedup source cols into gather SLOTS (a slot's
    gathered row can feed many output rows -> multi-hot S column, summed on
    host exactly like segment_sum). Slots sorted ascending (HBM page
    locality), padded per block to x128 and globally to xGATHER_B.
    Returns (iw, s8, tiles_per_block):
      iw: [ncalls, 128, GATHER_B//16] int16  dma_gather slot indices
      s8: [ncalls, 128, TPC*128] bf16        S^T tiles, S[slot, r] = sum v_e
    """
    rows = np.asarray(rows, np.int64)
    cols = np.asarray(cols, np.int64)
    vals = np.asarray(vals, np.float32)
    blk = rows >> 7
    order = np.argsort(blk, kind="stable")
    r_s, c_s, v_s = rows[order], cols[order], vals[order]
    blk = r_s >> 7
    counts = np.bincount(blk, minlength=NBLK)
    idx_p, s_p, tpb = [], [], []
    start = 0
    for I in range(NBLK):
        cnt = int(counts[I])
        sl = slice(start, start + cnt)
        start += cnt
        uc, inv = np.unique(c_s[sl], return_inverse=True)
        nsl = len(uc)
        npad = nsl + ((-nsl) % 128)
        S_blk = np.zeros((npad, 128), np.float32)
        np.add.at(S_blk, (inv, r_s[sl] - I * 128), v_s[sl])
        idx_p += [uc, np.zeros(npad - nsl, np.int64)]
        s_p.append(S_blk)
        tpb.append(npad // 128)
    idx = np.concatenate(idx_p)
    s_all = np.concatenate(s_p, 0)
    padt = (-len(idx)) % GATHER_B
    if padt:
        idx = np.concatenate([idx, np.zeros(padt, np.int64)])
        s_all = np.concatenate([s_all, np.zeros((padt, 128), np.float32)], 0)
        tpb[-1] += padt // 128
    E = len(idx)
    ncalls = E // GATHER_B
    # dma_gather index wrap: within a call, unwrapped[i] = wrap[i % 16, i // 16]
    iw = idx.reshape(ncalls, GATHER_B // 16, 16).transpose(0, 2, 1)
    iw = np.tile(iw, (1, NCORES, 1)).astype(np.int16)  # replicate to 128 parts
    s8 = (s_all.reshape(ncalls, TPC, 128, 128).transpose(0, 2, 1, 3)
          .reshape(ncalls, 128, TPC * 128).astype(ml_dtypes.bfloat16))
    return iw, np.ascontiguousarray(s8), tpb


def _prep_theta(Theta, bias):
    """Fold the Chebyshev affine + (c,m) interleave into K-tiles of the
    projection weight.  Feature order (K rows) is m-major over the effective
    matrices [x0, x1_0, z_0, x1_1, z_1] with z_s = A_s @ x1_s:
      m0 row c: Theta[c*5+0] - Theta[c*5+2] - Theta[c*5+4]
      m1 row c: Theta[c*5+1]
      m2 row c: 2*Theta[c*5+2]
      m3 row c: Theta[c*5+3]
      m4 row c: 2*Theta[c*5+4]
    """
    Theta = np.asarray(Theta, np.float64).reshape(C_IN, 5, C_OUT)  # [c, m, co]
    m = [Theta[:, i, :] for i in range(5)]
    th = np.stack([m[0] - m[2] - m[4], m[1], 2 * m[2], m[3], 2 * m[4]], 0)
    th0 = np.concatenate([th[0], th[1]], 0)   # [128, 64]
    th1 = np.concatenate([th[2], th[3]], 0)   # [128, 64]
    th2 = th[4]                                # [64, 64]
    b = np.asarray(bias, np.float64).reshape(1, C_OUT)
    c = ml_dtypes.bfloat16
    return th0.astype(c), th1.astype(c), th2.astype(c), b.astype(c)


# --------------------------------------------------------------------------
# device program (identical on all 8 cores; inputs differ per core)
# --------------------------------------------------------------------------
def _build_schedule(tpb):
    """Flat tile schedule: per global tile -> (block, first, last)."""
    sched = []
    for I, T in enumerate(tpb):
        for j in range(T):
            sched.append((I, j == 0, j == T - 1))
    return sched


def _emit_spmm(nc, tc, ctx, pools, src_ap, y_ap, yT_ap, iw_ap, s8_ap,
               sched, tag):
    """One spmm: y[r] += sum_e v_e * src[col_e]  (+ epilogue transposes)."""
    (iw_pool, vr_pool, g_pool, s_pool, y_pool, tr_pool,
     ps_y, ps_tr, const) = pools
    nreg = const["nreg"]
    iota_sb, ident_sb = const["iota"], const["ident"]
    ntiles = len(sched)
    ncalls = ntiles // TPC

    cur_psum = None
    G = None
    sc = None
    for t, (I, first, last) in enumerate(sched):
        c, g = divmod(t, TPC)
        if g == 0:
            it = iw_pool.tile([128, GATHER_B // 16], i16, tag="iw")
            nc.sync.dma_start(it[:], iw_ap[c])
            G = g_pool.tile([128, TPC, COLS], bf16, tag="G")
            nc.gpsimd.dma_gather(G[:], src_ap, it[:], GATHER_B, nreg,
                                 COLS, queue_num=c % 4)
            sc = s_pool.tile([128, TPC * 128], bf16, tag="S")
            nc.scalar.dma_start(sc[:], s8_ap[c])
        if first:
            cur_psum = ps_y.tile([128, COLS], f32, tag="psy")
        nc.tensor.matmul(cur_psum[:], sc[:, g * 128:(g + 1) * 128], G[:, g, :],
                         start=first, stop=last)
        if last:
            y_sb = y_pool.tile([128, COLS], bf16, tag="ysb")
            nc.vector.tensor_copy(y_sb[:], cur_psum[:])
            if y_ap is not None:
                nc.sync.dma_start(y_ap[I * 128:(I + 1) * 128, :], y_sb[:])
            y_v = y_sb[:].rearrange("p (c b) -> p b c", b=BLOC)
            for bl in range(0 if os.environ.get("KNOTR") else BLOC):
                pt = ps_tr.tile([C_IN, 128], bf16, tag="pstr")
                nc.tensor.transpose(pt[:], y_v[:, bl, :], ident_sb[:])
                sb_t = tr_pool.tile([C_IN, 128], bf16, tag="trsb")
                nc.scalar.copy(sb_t[:], pt[:])
                nc.scalar.dma_start(yT_ap[bl, I], sb_t[:])


def _build_program(iw_shapes, ntiles_s, tpb_s):
    """Build the full Bass/Tile program. Returns nc."""
    nc = bacc.Bacc("TRN2", target_bir_lowering=False, debug=False,
                   num_swdge_queues=4, dynamic_dma_scratch_size=98304)

    # inputs
    x0b = nc.dram_tensor("x0b", [NPAD, COLS], bf16, kind="ExternalInput").ap()
    x0T = nc.dram_tensor("x0T", [BLOC, NBLK, C_IN, 128], bf16,
                         kind="ExternalInput").ap()
    iw, s8 = [], []
    for s in range(N_SUPPORTS):
        iw.append(nc.dram_tensor(f"iw{s}", list(iw_shapes[s]), i16,
                                 kind="ExternalInput").ap())
        s8.append(nc.dram_tensor(f"s8{s}", [iw_shapes[s][0], 128, TPC * 128],
                                 bf16, kind="ExternalInput").ap())
    th0 = nc.dram_tensor("th0", [128, C_OUT], bf16, kind="ExternalInput").ap()
    th1 = nc.dram_tensor("th1", [128, C_OUT], bf16, kind="ExternalInput").ap()
    th2 = nc.dram_tensor("th2", [C_IN, C_OUT], bf16, kind="ExternalInput").ap()
    bias_d = nc.dram_tensor("biasd", [1, C_OUT], bf16, kind="ExternalInput").ap()
    iota_d = nc.dram_tensor("iota", [128, 128], bf16, kind="ExternalInput").ap()
    ident_d = nc.dram_tensor("ident", [128, 128], bf16,
                             kind="ExternalInput").ap()

    # internal DRAM
    y1 = [nc.dram_tensor(f"y1_{s}", [NPAD, COLS], bf16).ap()
          for s in range(N_SUPPORTS)]
    y1T = [nc.dram_tensor(f"y1T_{s}", [BLOC, NBLK, C_IN, 128], bf16).ap()
           for s in range(N_SUPPORTS)]
    zT = [nc.dram_tensor(f"zT_{s}", [BLOC, NBLK, C_IN, 128], bf16).ap()
          for s in range(N_SUPPORTS)]

    # output
    out_d = nc.dram_tensor("out", [BLOC, NBLK, 128, C_OUT], f32,
                           kind="ExternalOutput").ap()

    with tile.TileContext(nc) as tc, ExitStack() as ctx:
        const_p = ctx.enter_context(tc.tile_pool(name="const", bufs=1))
        iota_sb = const_p.tile([128, 128], bf16)
        nc.sync.dma_start(iota_sb[:], iota_d[:])
        ident_sb = const_p.tile([128, 128], bf16)
        nc.sync.dma_start(ident_sb[:], ident_d[:])
        th0_sb = const_p.tile([128, C_OUT], bf16)
        nc.sync.dma_start(th0_sb[:], th0[:])
        th1_sb = const_p.tile([128, C_OUT], bf16)
        nc.sync.dma_start(th1_sb[:], th1[:])
        th2_sb = const_p.tile([C_IN, C_OUT], bf16)
        nc.sync.dma_start(th2_sb[:], th2[:])
        bias_sb = const_p.tile([1, C_OUT], bf16)
        nc.sync.dma_start(bias_sb[:], bias_d[:])
        ones_sb = const_p.tile([1, 128], bf16)
        nc.vector.memset(ones_sb[:], 1.0)
        nreg = nc.gpsimd.to_reg(GATHER_B)
        const = {"iota": iota_sb, "ident": ident_sb, "nreg": nreg}

        iw_pool = ctx.enter_context(tc.tile_pool(name="iw", bufs=4))
        vr_pool = ctx.enter_context(tc.tile_pool(name="vr", bufs=2))
        g_pool = ctx.enter_context(tc.tile_pool(name="g", bufs=8))
        s_pool = ctx.enter_context(tc.tile_pool(name="s", bufs=6))
        y_pool = ctx.enter_context(tc.tile_pool(name="y", bufs=3))
        tr_pool = ctx.enter_context(tc.tile_pool(name="tr", bufs=4))
        ps_y = ctx.enter_context(tc.tile_pool(name="psy", bufs=3, space="PSUM"))
        ps_tr = ctx.enter_context(tc.tile_pool(name="pstr", bufs=3,
                                               space="PSUM"))
        ps_o = ctx.enter_context(tc.tile_pool(name="pso", bufs=2, space="PSUM"))
        xc_pool = ctx.enter_context(tc.tile_pool(name="xc", bufs=4))
        o_pool = ctx.enter_context(tc.tile_pool(name="o", bufs=3))

        pools = (iw_pool, vr_pool, g_pool, s_pool, y_pool, tr_pool,
                 ps_y, ps_tr, const)

        part = os.environ.get("KPART", "full")
        scheds = [_build_schedule(tpb_s[s]) for s in range(N_SUPPORTS)]
        n_sup1 = 1 if part in ("spmm1", "spmm1t") else N_SUPPORTS
        # x1_s = A_s @ x0  (write node-major + transposed)
        for s in range(n_sup1):
            _emit_spmm(nc, tc, ctx, pools, x0b, y1[s], y1T[s],
                       iw[s], s8[s], scheds[s], f"x1_{s}")
        # z_s = A_s @ x1_s (transposed only)
        if part not in ("spmm1", "spmm1t", "spmm2"):
            for s in range(N_SUPPORTS):
                _emit_spmm(nc, tc, ctx, pools, y1[s], None, zT[s],
                           iw[s], s8[s], scheds[s], f"z_{s}")

        # projection: out[bl, I] = Xcat^T.T @ Th  (+ bias via ones row)
        mats = [x0T, y1T[0], zT[0], y1T[1], zT[1]]
        if part in ("spmm1", "spmm1t", "spmm2", "spmm4"):
            # dummy projection source writes so the output is still produced
            mats = [x0T] * 5
        for bl in range(BLOC):
            for I in range(NBLK):
                xc0 = xc_pool.tile([128, 128], bf16, tag="xc0")
                nc.sync.dma_start(xc0[0:C_IN, :], mats[0][bl, I])
                nc.sync.dma_start(xc0[C_IN:128, :], mats[1][bl, I])
                xc1 = xc_pool.tile([128, 128], bf16, tag="xc1")
                nc.sync.dma_start(xc1[0:C_IN, :], mats[2][bl, I])
                nc.sync.dma_start(xc1[C_IN:128, :], mats[3][bl, I])
                xc2 = xc_pool.tile([C_IN, 128], bf16, tag="xc2")
                nc.sync.dma_start(xc2[:], mats[4][bl, I])
                po = ps_o.tile([128, C_OUT], f32, tag="pso")
                nc.tensor.matmul(po[:], xc0[:], th0_sb[:], start=True,
                                 stop=False)
                nc.tensor.matmul(po[:], xc1[:], th1_sb[:], start=False,
                                 stop=False)
                nc.tensor.matmul(po[:], xc2[:], th2_sb[:], start=False,
                                 stop=False)
                nc.tensor.matmul(po[:], ones_sb[:], bias_sb[:], start=False,
                                 stop=True)
                ob = o_pool.tile([128, C_OUT], f32, tag="ob")
                nc.scalar.copy(ob[:], po[:])
                nc.sync.dma_start(out_d[bl, I], ob[:])
    nc.compile()
    return nc


# --------------------------------------------------------------------------
# public entry point
# --------------------------------------------------------------------------
def kernel(x, edge_vals, Theta, bias, edge_rows, edge_cols):
    x = np.ascontiguousarray(np.asarray(x, np.float32))
    edge_vals = np.asarray(edge_vals, np.float32)
    edge_rows = np.asarray(edge_rows, np.int32)
    edge_cols = np.asarray(edge_cols, np.int32)

    # ---- host prep ----
    x0 = x.reshape(N_NODES, C_IN * BATCH).reshape(N_NODES, C_IN, BATCH)
    th0, th1, th2, bias_p = _prep_theta(Theta, bias)
    iota_np = np.asarray(np.broadcast_to(
        np.arange(128, dtype=np.float32), (128, 128)).astype(ml_dtypes.bfloat16))
    ident_np = np.eye(128, dtype=ml_dtypes.bfloat16)

    iw_s, s8_s, tpb_s = [], [], []
    for s in range(N_SUPPORTS):
        iw, s8, tpb = _prep_edges(edge_rows[s], edge_cols[s], edge_vals[s])
        iw_s.append(iw); s8_s.append(s8); tpb_s.append(tpb)

    nc = _build_program([a.shape for a in iw_s], None, tpb_s)

    in_maps = []
    for k in range(NCORES):
        xk = x0[:, :, k * BLOC:(k + 1) * BLOC].reshape(N_NODES, COLS)
        xkp = np.zeros((NPAD, COLS), np.float32)
        xkp[:N_NODES] = xk
        x0b_np = xkp.astype(ml_dtypes.bfloat16)
        # x0T[bl, I, c, j] = xk[I*128+j, c*BLOC+bl]
        x0T_np = np.ascontiguousarray(
            xkp.reshape(NBLK, 128, C_IN, BLOC).transpose(3, 0, 2, 1)
        ).astype(ml_dtypes.bfloat16)
        im = {"x0b": np.asarray(x0b_np), "x0T": np.asarray(x0T_np),
              "th0": np.asarray(th0), "th1": np.asarray(th1),
              "th2": np.asarray(th2), "biasd": np.asarray(bias_p),
              "iota": iota_np, "ident": np.asarray(ident_np)}
        for s in range(N_SUPPORTS):
            im[f"iw{s}"] = iw_s[s]
            im[f"s8{s}"] = s8_s[s]
        in_maps.append(im)

    results = _run_pjrt(nc, in_maps)

    # ---- host assembly ----
    out = np.empty((BATCH, N_NODES, C_OUT), np.float32)
    for k in range(NCORES):
        ok = results[k]["out"]  # [BLOC, NBLK, 128, C_OUT]
        ok = ok.reshape(BLOC, NPAD, C_OUT)[:, :N_NODES, :]
        out[k * BLOC:(k + 1) * BLOC] = ok
    return out


# --------------------------------------------------------------------------
# PJRT execution (axon) — vendored from bass2jax.run_bass_via_pjrt, but
# without output-buffer donation so the compiled executable can be
# re-dispatched for timing (our kernel fully writes its output tensor).
# --------------------------------------------------------------------------
def _run_pjrt(nc, in_maps):
    import jax
    from jax.sharding import Mesh, PartitionSpec, NamedSharding
    from jax.experimental.shard_map import shard_map
    from concourse import bass2jax
    from concourse import mybir as mb

    bass2jax.install_neuronx_cc_hook()
    n_cores = len(in_maps)
    partition_name = (nc.partition_id_tensor.name
                      if nc.partition_id_tensor else None)

    in_names, out_names, out_avals, zero_outs = [], [], [], []
    for alloc in nc.m.functions[0].allocations:
        if not isinstance(alloc, mb.MemoryLocationSet):
            continue
        name = alloc.memorylocations[0].name
        if alloc.kind == "ExternalInput":
            if name != partition_name:
                in_names.append(name)
        elif alloc.kind == "ExternalOutput":
            out_names.append(name)
            shape = tuple(alloc.tensor_shape)
            dtype = mb.dt.np(alloc.dtype)
            out_avals.append(jax.core.ShapedArray(shape, dtype))
            zero_outs.append(np.zeros(shape, dtype))
    n_params = len(in_names)
    in_names.extend(out_names)
    if partition_name is not None:
        in_names.append(partition_name)

    def _body(*args):
        operands = list(args)
        if partition_name is not None:
            operands.append(bass2jax.partition_id_tensor())
        outs = bass2jax._bass_exec_p.bind(
            *operands,
            out_avals=tuple(out_avals),
            in_names=tuple(in_names),
            out_names=tuple(out_names),
            lowering_input_output_aliases=(),
            sim_require_finite=True,
            sim_require_nnan=True,
            nc=nc,
        )
        return tuple(outs)

    devices = jax.devices()[:n_cores]
    mesh = Mesh(np.asarray(devices), ("core",))
    in_specs = (PartitionSpec("core"),) * (n_params + len(out_names))
    out_specs = (PartitionSpec("core"),) * len(out_names)
    sharded = jax.jit(
        shard_map(_body, mesh=mesh, in_specs=in_specs, out_specs=out_specs,
                  check_rep=False),
        keep_unused=True,
    )
    per_core = [[np.asarray(m[name]) for name in in_names[:n_params]]
                for m in in_maps]
    sh = NamedSharding(mesh, PartitionSpec("core"))
    concat_in = [
        jax.device_put(
            np.concatenate([per_core[c][i] for c in range(n_cores)], axis=0),
            sh)
        for i in range(n_params)
    ]
    concat_zeros = [
        jax.device_put(np.zeros((n_cores * z.shape[0], *z.shape[1:]), z.dtype),
                       sh)
        for z in zero_outs
    ]
    out_arrs = sharded(*concat_in, *concat_zeros)
    jax.block_until_ready(out_arrs)
    LAST_RESULT["runner"] = (sharded, concat_in, concat_zeros)
    return [
        {name: np.asarray(out_arrs[i]).reshape(n_cores, *out_avals[i].shape)[c]
         for i, name in enumerate(out_names)}
        for c in range(n_cores)
    ]


def time_kernel(repeats=8):
    """Per-execution device time via queued-dispatch slope (ns)."""
    import jax
    import time
    sharded, concat_in, concat_zeros = LAST_RESULT["runner"]

    def run_n(n):
        t0 = time.perf_counter()
        outs = [sharded(*concat_in, *concat_zeros) for _ in range(n)]
        jax.block_until_ready(outs)
        return time.perf_counter() - t0

    run_n(1)  # warm
    t1 = min(run_n(1) for _ in range(3))
    tn = min(run_n(1 + repeats) for _ in range(2))
    dt = (tn - t1) / repeats
    LAST_RESULT["t1_s"] = t1
    LAST_RESULT["tn_s"] = tn
    return dt * 1e9

